# revision 10
# baseline (speedup 1.0000x reference)
"""GCN autoencoder (2-layer GCN + inner-product decoder) on 8 Trainium2
NeuronCores.

recon = A@(relu(A@(X W1)) W2) ; out = Z Z^T   with A[dst,src] += edge_w.

Sharding: nodes (rows of X/H1/Z and rows of the 8192x8192 output) are
split 1024-per-core; the edge list is partitioned by destination node,
which materializes as per-core column slices of the dense A^T (A is
0.4% sparse, but TensorE dense tiles beat gather/scatter DMA at this
size).  W1/W2 replicated.  X is sharded by node (1 MB/core instead of
8 MB replicated); an AllGather shares XW1 before layer 1's aggregation,
two more share Y = H1@W2 and Z^T.  Every AllGather is issued in two
halves so its latency hides behind the next compute stage, which walks
the contraction dim in half-order (PSUM accumulation is order-free).
Bulk loads (A^T, X) ride the sync-engine DGE ring together with the
gather-in copies; bounce-outs use the scalar-engine ring so they are
never head-of-line blocked behind megabyte loads.

Precision: A^T is stored as fp8-e3m4 scaled by 4 (edge weights are
uniform [0,1); x4 puts them in e3m4's normal range, ~1.2% per-entry
rounding).  TensorE takes the fp8 tile directly as the moving operand
of bf16 matmuls (both sides upconvert to FP22 internally), so the only
cost is the quantization itself; the scale is folded into W2 and the
Z^T copy.  Everything else is bf16 with fp32 PSUM accumulation.  The
8192x8192 output is stored bf16 and upcast to f32 on the host.
"""

import os

import ml_dtypes
import numpy as np

N_NODES = 8192
N_CORES = 8
ROWS = N_NODES // N_CORES          # 1024 output rows per core
KT = N_NODES // 128                # 64 k-tiles over the node dim
KPC = KT // N_CORES                # 8 k-tiles owned per core
D_IN, D_H, D_Z = 512, 256, 64

BF16 = ml_dtypes.bfloat16
E3M4 = ml_dtypes.float8_e3m4
A_SCALE = 4.0

_COMPILED = None        # cached (nc, meta) across kernel() calls
LAST_EXEC_TIME_NS = None
LAST_RESULTS = None


def _patch_tile_drain():
    """This container's walrus build rejects instructions carrying more
    than one sync-wait command (it lacks the multi-wait lowering).  Two
    fixes, both inside Tile's teardown:

    1. A legalization pass over every traced instruction: extra waits
       are hoisted onto fresh single-wait nops inserted just before the
       instruction on the same engine (same-engine sequencing preserves
       semantics).
    2. The kernel-tail drain (one wait per live semaphore) is split the
       same way.
    """
    import concourse.mybir as mybir
    import concourse.tile as tile
    from bass_rust import ScopedClock

    def _split_multi_waits(nc):
        f = nc.m.functions[0]
        for blk in f.blocks:
            insts = list(blk.instructions)
            if not any(
                i.sync_info is not None and len(i.sync_info.on_wait) > 1
                for i in insts
            ):
                continue
            new_list = []
            for inst in insts:
                si = inst.sync_info
                if si is not None and len(si.on_wait) > 1 and inst.engine in nc.engines:
                    waits = list(si.on_wait)
                    eng = nc.engines[inst.engine]
                    for w in waits[:-1]:
                        n = eng.nop(nofuse=True, hint="wsplit")
                        # the builder appended it to cur_bb; relocate
                        cb = nc.cur_bb.bb
                        cur = cb.instructions
                        assert cur and cur[-1].name == n.ins.name
                        cur.pop()
                        cb.instructions = cur
                        n.ins.sync_info = mybir.SyncInfo(
                            on_wait=[w], on_update=[]
                        )
                        new_list.append(n.ins)
                    inst.sync_info = mybir.SyncInfo(
                        on_wait=[waits[-1]], on_update=list(si.on_update)
                    )
                new_list.append(inst)
            blk.instructions = new_list

    def _drain_and_barrier(self, tick_clock, wait_clock):
        nc = self.nc
        _split_multi_waits(nc)
        probe = nc.sync.nop(nofuse=True, hint="drain_waits")
        wait_clock.add_sem_waits(
            probe.ins, ScopedClock({None: tick_clock.global_clock})
        )
        waits = list(probe.ins.sync_info.on_wait) if probe.ins.sync_info else []
        if len(waits) > 1:
            probe.ins.sync_info = mybir.SyncInfo(on_wait=[waits[0]], on_update=[])
            for w in waits[1:]:
                n = nc.sync.nop(nofuse=True, hint="drain_waits")
                n.ins.sync_info = mybir.SyncInfo(on_wait=[w], on_update=[])
        nc.sync.drain()
        nc.all_engine_barrier()
        assert self.sems is not None
        popped = nc._tile_sem_poison_stack.pop()
        assert popped is self._sem_poison
        nc.clear_and_free_semaphores(list(self.sems.allocated().values()))
        nc.all_engine_barrier()

    tile.TileContext._drain_and_barrier = _drain_and_barrier


def _build_program():
    import concourse.bass as bass
    import concourse.mybir as mybir
    import concourse.tile as tile

    _patch_tile_drain()

    nc = bass.Bass(num_devices=N_CORES)
    bf = mybir.dt.bfloat16
    f8 = mybir.dt.float8e3
    f32 = mybir.dt.float32

    at_in = nc.dram_tensor("at_in", [128, KT, ROWS], f8, kind="ExternalInput")
    xt_in = nc.dram_tensor("xt_in", [128, D_IN // 128, ROWS], bf,
                           kind="ExternalInput")
    w1_in = nc.dram_tensor("w1_in", [128, D_IN // 128, D_H], bf,
                           kind="ExternalInput")
    w2_in = nc.dram_tensor("w2_in", [128, D_H // 128, D_Z], bf,
                           kind="ExternalInput")
    # column dim factored (rank, half, 512) so stage 5 can stream out
    # per collective-half stripes; host reshapes back to [ROWS, N_NODES]
    recon_out = nc.dram_tensor("recon", [ROWS, N_CORES, 2, 512], bf,
                               kind="ExternalOutput")

    groups = [list(range(N_CORES))]
    MB = ROWS // 128   # 8 row blocks per core
    HB = MB // 2       # 4 row blocks per collective half

    with tile.TileContext(nc) as tc:
        with (
            tc.tile_pool(name="dram", bufs=1, space="DRAM") as dram,
            tc.tile_pool(name="big", bufs=1) as big,
            tc.tile_pool(name="outp", bufs=2) as outp,
        ):
            # ---- collective bounce buffers (DRAM), one pair per half ----
            xw1_mine_d = [dram.tile([128, HB, D_H], bf, name=f"xw1m{h}")
                          for h in range(2)]
            xw1_full_d = [dram.tile([N_CORES, 128, HB, D_H], bf,
                                    addr_space="Shared", name=f"xw1f{h}")
                          for h in range(2)]
            y_mine_d = [dram.tile([128, HB, D_Z], bf, name=f"ym{h}")
                        for h in range(2)]
            y_full_d = [dram.tile([N_CORES, 128, HB, D_Z], bf,
                                  addr_space="Shared", name=f"yf{h}")
                        for h in range(2)]
            zt_mine_d = [dram.tile([D_Z, 512], bf, name=f"ztm{h}")
                         for h in range(2)]
            zt_full_d = [dram.tile([N_CORES, D_Z, 512], bf,
                                   addr_space="Shared", name=f"ztf{h}")
                         for h in range(2)]
            warm_m_d = dram.tile([1, 32], bf, name="warm_m")
            warm_f_d = dram.tile([N_CORES, 1, 32], bf, addr_space="Shared",
                                 name="warm_f")

            # ---- resident SBUF tiles ----
            at_sb = big.tile([128, KT, ROWS], f8, name="at_sb")      # 64 KB/part
            xw1_sb = big.tile([128, KT, D_H], bf, name="xw1_sb")     # 32 KB/part
            xw1my_sb = big.tile([128, MB, D_H], bf, name="xw1my_sb")
            h1t_sb = big.tile([128, D_H // 128, ROWS], bf, name="h1t_sb")
            y_stage = big.tile([128, MB, D_Z], bf, name="y_stage")
            y_sb = big.tile([128, KT, D_Z], bf, name="y_sb")
            ztm_sb = big.tile([D_Z, ROWS], bf, name="ztm_sb")
            ztf_sb = big.tile([D_Z, N_CORES, ROWS], bf, name="ztf_sb")
            xt_sb = big.tile([128, D_IN // 128, ROWS], bf, name="xt_sb")
            w1_sb = big.tile([128, D_IN // 128, D_H], bf, name="w1_sb")
            w2_sb = big.tile([128, D_H // 128, D_Z], bf, name="w2_sb")
            bias_sb = big.tile([128, 1], f32, name="bias_sb")
            nc.vector.memset(bias_sb[:], 0.0)
            nc.sync.dma_start(out=xt_sb[:], in_=xt_in[:])
            nc.sync.dma_start(out=w1_sb[:], in_=w1_in[:])
            nc.sync.dma_start(out=w2_sb[:], in_=w2_in[:])
            # A^T bulk load; shares the sync ring with the gather-ins
            # issued below (those wait on their collective, but already-
            # queued transfers keep draining ahead of them).
            for c in range(8):
                ks = slice(c * (KT // 8), (c + 1) * (KT // 8))
                nc.sync.dma_start(out=at_sb[:, ks, :], in_=at_in[:, ks, :])

            # Warm-up collective: the first collective of a NEFF pays
            # ~25-40us of one-time stack latency (measured: doorbells at
            # ~30us, data moving only at ~62-77us).  Fire a 64-byte
            # AllGather immediately so that cost runs concurrently with
            # stage 1 and the A^T load instead of serializing AG#0.
            nc.gpsimd.collective_compute(
                "AllGather", mybir.AluOpType.bypass, replica_groups=groups,
                ins=[warm_m_d[:]], outs=[warm_f_d[:]],
            )

            copy_engines = [nc.vector, nc.scalar]

            def copy_to(idx, dst, src, engines=copy_engines):
                eng = engines[idx % len(engines)]
                if eng is nc.scalar:
                    nc.scalar.activation(
                        dst, src, mybir.ActivationFunctionType.Copy
                    )
                else:
                    eng.tensor_copy(dst, src)

            # Half-order walk of the 64 k-tiles: half h covers blocks
            # h*4..h*4+3 of every rank, i.e. exactly what AllGather half
            # h of the producer stage delivered.
            def half_ks(h):
                return [j * KPC + h * HB + b for j in range(N_CORES)
                        for b in range(HB)]

            # ---- stage 1: XW1 for my 1024 nodes + AllGather (halved) ----
            with tc.tile_pool(name="ps1", bufs=2, space="PSUM") as ps1:
                for h in range(2):
                    for b in range(h * HB, h * HB + HB):
                        acc = ps1.tile([128, D_H], f32, tag="ps", name="acc_xw1")
                        for kk in range(D_IN // 128):
                            nc.tensor.matmul(
                                acc[:],
                                xt_sb[:, kk, b * 128:(b + 1) * 128],
                                w1_sb[:, kk, :],
                                start=(kk == 0),
                                stop=(kk == D_IN // 128 - 1),
                            )
                        copy_to(b, xw1my_sb[:, b, :], acc[:])
                    nc.scalar.dma_start(
                        out=xw1_mine_d[h][:],
                        in_=xw1my_sb[:, h * HB:h * HB + HB, :])
                    nc.gpsimd.collective_compute(
                        "AllGather", mybir.AluOpType.bypass,
                        replica_groups=groups,
                        ins=[xw1_mine_d[h][:]], outs=[xw1_full_d[h][:]],
                    )
                    for j in range(N_CORES):
                        nc.sync.dma_start(
                            out=xw1_sb[:, j * KPC + h * HB:
                                       j * KPC + h * HB + HB, :],
                            in_=xw1_full_d[h][j])

            # ---- stage 2: H1^T = relu(XW1^T @ A_i^T), feat-major.
            # Half-major k walk keeps 4 PSUM groups open so compute on
            # half 0 hides the half-1 AllGather.
            with tc.tile_pool(name="ps2", bufs=4, space="PSUM") as ps2:
                accs = {}
                for fb in range(D_H // 128):
                    for nb in range(ROWS // 512):
                        accs[fb, nb] = ps2.tile([128, 512], f32, tag="ps",
                                                name=f"acc_h1_{fb}{nb}")
                ks = half_ks(0) + half_ks(1)
                for ki, k in enumerate(ks):
                    for fb in range(D_H // 128):
                        for nb in range(ROWS // 512):
                            nc.tensor.matmul(
                                accs[fb, nb][:],
                                xw1_sb[:, k, fb * 128:(fb + 1) * 128],
                                at_sb[:, k, nb * 512:(nb + 1) * 512],
                                start=(ki == 0),
                                stop=(ki == KT - 1),
                            )
                for fb in range(D_H // 128):
                    for nb in range(ROWS // 512):
                        nc.scalar.activation(
                            h1t_sb[:, fb, nb * 512:(nb + 1) * 512],
                            accs[fb, nb][:],
                            mybir.ActivationFunctionType.Relu, bias=bias_sb[:],
                        )

            # ---- stage 3: Y = H1 @ W2 (node-major) + AllGather (halved) ----
            with tc.tile_pool(name="ps3", bufs=2, space="PSUM") as ps3:
                for h in range(2):
                    for b in range(h * HB, h * HB + HB):
                        acc = ps3.tile([128, D_Z], f32, tag="ps", name="acc_y")
                        for fb in range(D_H // 128):
                            nc.tensor.matmul(
                                acc[:],
                                h1t_sb[:, fb, b * 128:(b + 1) * 128],
                                w2_sb[:, fb, :],
                                start=(fb == 0),
                                stop=(fb == D_H // 128 - 1),
                            )
                        nc.vector.tensor_copy(y_stage[:, b, :], acc[:])
                    nc.scalar.dma_start(
                        out=y_mine_d[h][:],
                        in_=y_stage[:, h * HB:h * HB + HB, :])
                    nc.gpsimd.collective_compute(
                        "AllGather", mybir.AluOpType.bypass,
                        replica_groups=groups,
                        ins=[y_mine_d[h][:]], outs=[y_full_d[h][:]],
                    )
                    for j in range(N_CORES):
                        nc.sync.dma_start(
                            out=y_sb[:, j * KPC + h * HB:
                                     j * KPC + h * HB + HB, :],
                            in_=y_full_d[h][j])

            # ---- stage 4: Z^T = Y^T @ A_i^T + AllGather (halved by col) ----
            with tc.tile_pool(name="ps4", bufs=2, space="PSUM") as ps4:
                ks = half_ks(0) + half_ks(1)
                for nb in range(ROWS // 512):
                    acc = ps4.tile([D_Z, 512], f32, tag="ps", name="acc_zt")
                    for ki, k in enumerate(ks):
                        nc.tensor.matmul(
                            acc[:],
                            y_sb[:, k, :],
                            at_sb[:, k, nb * 512:(nb + 1) * 512],
                            start=(ki == 0),
                            stop=(ki == KT - 1),
                        )
                    # undo the fp8 A^T pre-scale (Y already absorbed one
                    # factor via W2; this is the second A application)
                    nc.scalar.activation(
                        ztm_sb[:, nb * 512:(nb + 1) * 512], acc[:],
                        mybir.ActivationFunctionType.Copy,
                        scale=1.0 / A_SCALE,
                    )
                    nc.scalar.dma_start(
                        out=zt_mine_d[nb][:],
                        in_=ztm_sb[:, nb * 512:(nb + 1) * 512])
                    nc.gpsimd.collective_compute(
                        "AllGather", mybir.AluOpType.bypass,
                        replica_groups=groups,
                        ins=[zt_mine_d[nb][:]], outs=[zt_full_d[nb][:]],
                    )
                    for j in range(N_CORES):
                        nc.sync.dma_start(
                            out=ztf_sb[:, j, nb * 512:(nb + 1) * 512],
                            in_=zt_full_d[nb][j])

            # ---- stage 5: recon_i = Z_i @ Z^T, bf16 out.
            # Column chunks walk half 0 of every rank first so the
            # half-1 Z^T AllGather hides behind them.
            ci = 0
            out_dge = [nc.sync, nc.scalar]
            with tc.tile_pool(name="ps5", bufs=4, space="PSUM") as ps5:
                for b in range(MB):
                    ot = outp.tile([128, N_CORES, 2, 512], bf, tag="out",
                                   name="out_sb")
                    for h in range(2):
                        for j in range(N_CORES):
                            acc = ps5.tile([128, 512], f32, tag="ps",
                                           name="acc_rec")
                            nc.tensor.matmul(
                                acc[:],
                                ztm_sb[:, b * 128:(b + 1) * 128],
                                ztf_sb[:, j, h * 512:(h + 1) * 512],
                                start=True, stop=True,
                            )
                            copy_to(ci, ot[:, j, h, :], acc[:])
                            ci += 1
                        # stream this half's stripes out as soon as its
                        # 8 chunks land; rings alternate so writes drain
                        # in parallel
                        out_dge[(2 * b + h) % 2].dma_start(
                            out=recon_out[b * 128:(b + 1) * 128, :, h, :],
                            in_=ot[:, :, h, :],
                        )

    nc.finalize()
    return nc


def _prep_inputs(x, edge_src, edge_dst, edge_w, W1, W2):
    """Host-side: dense A^T (fp8), per-core slices, PE-friendly layouts."""
    at = np.zeros((N_NODES, N_NODES), dtype=np.float32)     # [src, dst]
    np.add.at(at, (edge_src, edge_dst), edge_w)
    at *= A_SCALE
    np.minimum(at, 15.5, out=at)     # e3m4 max normal (dup edges can sum)

    xt = np.ascontiguousarray(x.T)                          # [feat, node]
    w1_dev = np.ascontiguousarray(
        W1.reshape(D_IN // 128, 128, D_H).transpose(1, 0, 2)).astype(BF16)
    w2_dev = np.ascontiguousarray(
        (W2 / A_SCALE).reshape(D_H // 128, 128, D_Z).transpose(1, 0, 2)
    ).astype(BF16)

    in_maps = []
    for i in range(N_CORES):
        cols = slice(i * ROWS, (i + 1) * ROWS)
        at_i = np.ascontiguousarray(
            at[:, cols].reshape(KT, 128, ROWS).transpose(1, 0, 2)).astype(E3M4)
        xt_i = np.ascontiguousarray(
            xt[:, cols].reshape(D_IN // 128, 128, ROWS).transpose(1, 0, 2)
        ).astype(BF16)
        in_maps.append({
            "at_in": at_i, "xt_in": xt_i, "w1_in": w1_dev, "w2_in": w2_dev,
        })
    return in_maps


def kernel(x, edge_src, edge_dst, edge_w, W1, W2):
    global _COMPILED, LAST_EXEC_TIME_NS, LAST_RESULTS
    from concourse.bass_utils import run_bass_kernel_spmd

    if _COMPILED is None:
        _COMPILED = _build_program()
    nc = _COMPILED

    in_maps = _prep_inputs(
        np.asarray(x, dtype=np.float32),
        np.asarray(edge_src), np.asarray(edge_dst),
        np.asarray(edge_w, dtype=np.float32),
        np.asarray(W1, dtype=np.float32), np.asarray(W2, dtype=np.float32),
    )

    trace = bool(int(os.environ.get("KERNEL_TRACE", "0")))
    res = run_bass_kernel_spmd(
        nc, in_maps, list(range(N_CORES)), trace=trace,
    )
    LAST_RESULTS = res
    LAST_EXEC_TIME_NS = res.exec_time_ns
    return np.concatenate(
        [res.results[i]["recon"].reshape(ROWS, N_NODES).astype(np.float32)
         for i in range(N_CORES)],
        axis=0,
    )


# revision 16
# speedup vs baseline: 1.0467x; 1.0467x over previous
"""GCN autoencoder (2-layer GCN + inner-product decoder) on 8 Trainium2
NeuronCores.

recon = A@(relu(A@(X W1)) W2) ; out = Z Z^T   with A[dst,src] += edge_w.

Sharding: nodes (rows of X/H1/Z and rows of the 8192x8192 output) are
split 1024-per-core; the edge list is partitioned by destination node,
which materializes as per-core column slices of the dense A^T (A is
0.4% sparse, but TensorE dense tiles beat gather/scatter DMA at this
size).  W1/W2 replicated.  X is sharded by node (1 MB/core instead of
8 MB replicated); an AllGather shares XW1 before layer 1's aggregation,
two more share Y = H1@W2 and Z^T.  Every AllGather is issued in two
halves so its latency hides behind the next compute stage, which walks
the contraction dim in half-order (PSUM accumulation is order-free).
Bulk loads (A^T, X) ride the sync-engine DGE ring together with the
gather-in copies; bounce-outs use the scalar-engine ring so they are
never head-of-line blocked behind megabyte loads.

Precision: A^T is stored as fp8-e3m4 scaled by 4 (edge weights are
uniform [0,1); x4 puts them in e3m4's normal range, ~1.2% per-entry
rounding).  TensorE takes the fp8 tile directly as the moving operand
of bf16 matmuls (both sides upconvert to FP22 internally), so the only
cost is the quantization itself; the scale is folded into W2 and the
Z^T copy.  Everything else is bf16 with fp32 PSUM accumulation.  The
8192x8192 output is stored bf16 and upcast to f32 on the host.
"""

import os

import ml_dtypes
import numpy as np

N_NODES = 8192
N_CORES = 8
ROWS = N_NODES // N_CORES          # 1024 output rows per core
KT = N_NODES // 128                # 64 k-tiles over the node dim
KPC = KT // N_CORES                # 8 k-tiles owned per core
D_IN, D_H, D_Z = 512, 256, 64

BF16 = ml_dtypes.bfloat16
E3M4 = ml_dtypes.float8_e3m4
A_SCALE = 4.0

_COMPILED = None        # cached (nc, meta) across kernel() calls
LAST_EXEC_TIME_NS = None
LAST_RESULTS = None


def _patch_tile_drain():
    """This container's walrus build rejects instructions carrying more
    than one sync-wait command (it lacks the multi-wait lowering).  Two
    fixes, both inside Tile's teardown:

    1. A legalization pass over every traced instruction: extra waits
       are hoisted onto fresh single-wait nops inserted just before the
       instruction on the same engine (same-engine sequencing preserves
       semantics).
    2. The kernel-tail drain (one wait per live semaphore) is split the
       same way.
    """
    import concourse.mybir as mybir
    import concourse.tile as tile
    from bass_rust import ScopedClock

    def _split_multi_waits(nc):
        f = nc.m.functions[0]
        for blk in f.blocks:
            insts = list(blk.instructions)
            if not any(
                i.sync_info is not None and len(i.sync_info.on_wait) > 1
                for i in insts
            ):
                continue
            new_list = []
            for inst in insts:
                si = inst.sync_info
                if si is not None and len(si.on_wait) > 1 and inst.engine in nc.engines:
                    waits = list(si.on_wait)
                    eng = nc.engines[inst.engine]
                    for w in waits[:-1]:
                        n = eng.nop(nofuse=True, hint="wsplit")
                        # the builder appended it to cur_bb; relocate
                        cb = nc.cur_bb.bb
                        cur = cb.instructions
                        assert cur and cur[-1].name == n.ins.name
                        cur.pop()
                        cb.instructions = cur
                        n.ins.sync_info = mybir.SyncInfo(
                            on_wait=[w], on_update=[]
                        )
                        new_list.append(n.ins)
                    inst.sync_info = mybir.SyncInfo(
                        on_wait=[waits[-1]], on_update=list(si.on_update)
                    )
                new_list.append(inst)
            blk.instructions = new_list

    def _drain_and_barrier(self, tick_clock, wait_clock):
        nc = self.nc
        _split_multi_waits(nc)
        probe = nc.sync.nop(nofuse=True, hint="drain_waits")
        wait_clock.add_sem_waits(
            probe.ins, ScopedClock({None: tick_clock.global_clock})
        )
        waits = list(probe.ins.sync_info.on_wait) if probe.ins.sync_info else []
        if len(waits) > 1:
            probe.ins.sync_info = mybir.SyncInfo(on_wait=[waits[0]], on_update=[])
            for w in waits[1:]:
                n = nc.sync.nop(nofuse=True, hint="drain_waits")
                n.ins.sync_info = mybir.SyncInfo(on_wait=[w], on_update=[])
        nc.sync.drain()
        nc.all_engine_barrier()
        assert self.sems is not None
        popped = nc._tile_sem_poison_stack.pop()
        assert popped is self._sem_poison
        nc.clear_and_free_semaphores(list(self.sems.allocated().values()))
        nc.all_engine_barrier()

    tile.TileContext._drain_and_barrier = _drain_and_barrier


def _build_program():
    import concourse.bass as bass
    import concourse.mybir as mybir
    import concourse.tile as tile

    _patch_tile_drain()

    nc = bass.Bass(num_devices=N_CORES)
    bf = mybir.dt.bfloat16
    f8 = mybir.dt.float8e3
    f32 = mybir.dt.float32

    at_in = nc.dram_tensor("at_in", [128, KT, ROWS], f8, kind="ExternalInput")
    xt_in = nc.dram_tensor("xt_in", [128, D_IN // 128, ROWS], bf,
                           kind="ExternalInput")
    w1_in = nc.dram_tensor("w1_in", [128, D_IN // 128, D_H], bf,
                           kind="ExternalInput")
    w2_in = nc.dram_tensor("w2_in", [128, D_H // 128, D_Z], bf,
                           kind="ExternalInput")
    recon_out = nc.dram_tensor("recon", [ROWS, N_NODES], bf, kind="ExternalOutput")

    groups = [list(range(N_CORES))]
    MB = ROWS // 128   # 8 row blocks per core
    HB = MB // 2       # 4 row blocks per collective half

    with tile.TileContext(nc) as tc:
        with (
            tc.tile_pool(name="dram", bufs=1, space="DRAM") as dram,
            tc.tile_pool(name="big", bufs=1) as big,
            tc.tile_pool(name="outp", bufs=2) as outp,
        ):
            # ---- collective bounce buffers (DRAM), one pair per half ----
            xw1_mine_d = [dram.tile([128, HB, D_H], bf, name=f"xw1m{h}")
                          for h in range(2)]
            xw1_full_d = [dram.tile([N_CORES, 128, HB, D_H], bf,
                                    addr_space="Shared", name=f"xw1f{h}")
                          for h in range(2)]
            y_mine_d = [dram.tile([128, HB, D_Z], bf, name=f"ym{h}")
                        for h in range(2)]
            y_full_d = [dram.tile([N_CORES, 128, HB, D_Z], bf,
                                  addr_space="Shared", name=f"yf{h}")
                        for h in range(2)]
            zt_mine_d = dram.tile([D_Z, ROWS], bf, name="zt_mine_d")
            zt_full_d = dram.tile([N_CORES, D_Z, ROWS], bf,
                                  addr_space="Shared", name="zt_full_d")

            # ---- resident SBUF tiles ----
            at_sb = big.tile([128, KT, ROWS], f8, name="at_sb")      # 64 KB/part
            xw1_sb = big.tile([128, KT, D_H], bf, name="xw1_sb")     # 32 KB/part
            xw1my_sb = big.tile([128, MB, D_H], bf, name="xw1my_sb")
            h1t_sb = big.tile([128, D_H // 128, ROWS], bf, name="h1t_sb")
            y_stage = big.tile([128, MB, D_Z], bf, name="y_stage")
            y_sb = big.tile([128, KT, D_Z], bf, name="y_sb")
            ztm_sb = big.tile([D_Z, ROWS], bf, name="ztm_sb")
            ztf_sb = big.tile([D_Z, N_CORES, ROWS], bf, name="ztf_sb")
            xt_sb = big.tile([128, D_IN // 128, ROWS], bf, name="xt_sb")
            w1_sb = big.tile([128, D_IN // 128, D_H], bf, name="w1_sb")
            w2_sb = big.tile([128, D_H // 128, D_Z], bf, name="w2_sb")
            bias_sb = big.tile([128, 1], f32, name="bias_sb")
            nc.vector.memset(bias_sb[:], 0.0)
            nc.sync.dma_start(out=w1_sb[:], in_=w1_in[:])
            # xt in halves so stage 1 starts after the first 512 nodes land
            nc.sync.dma_start(out=xt_sb[:, :, 0:512], in_=xt_in[:, :, 0:512])
            nc.sync.dma_start(out=xt_sb[:, :, 512:1024], in_=xt_in[:, :, 512:1024])
            nc.sync.dma_start(out=w2_sb[:], in_=w2_in[:])
            # A^T bulk load; shares the sync ring with the gather-ins
            # issued below (those wait on their collective, but already-
            # queued transfers keep draining ahead of them).
            for c in range(8):
                ks = slice(c * (KT // 8), (c + 1) * (KT // 8))
                nc.sync.dma_start(out=at_sb[:, ks, :], in_=at_in[:, ks, :])

            copy_engines = [nc.vector, nc.scalar]

            def copy_to(idx, dst, src, engines=copy_engines):
                eng = engines[idx % len(engines)]
                if eng is nc.scalar:
                    nc.scalar.activation(
                        dst, src, mybir.ActivationFunctionType.Copy
                    )
                else:
                    eng.tensor_copy(dst, src)

            # Half-order walk of the 64 k-tiles: half h covers blocks
            # h*4..h*4+3 of every rank, i.e. exactly what AllGather half
            # h of the producer stage delivered.
            def half_ks(h):
                return [j * KPC + h * HB + b for j in range(N_CORES)
                        for b in range(HB)]

            # ---- stage 1: XW1 for my 1024 nodes + AllGather (halved) ----
            with tc.tile_pool(name="ps1", bufs=2, space="PSUM") as ps1:
                for h in range(2):
                    for b in range(h * HB, h * HB + HB):
                        acc = ps1.tile([128, D_H], f32, tag="ps", name="acc_xw1")
                        for kk in range(D_IN // 128):
                            nc.tensor.matmul(
                                acc[:],
                                xt_sb[:, kk, b * 128:(b + 1) * 128],
                                w1_sb[:, kk, :],
                                start=(kk == 0),
                                stop=(kk == D_IN // 128 - 1),
                            )
                        copy_to(b, xw1my_sb[:, b, :], acc[:])
                    nc.scalar.dma_start(
                        out=xw1_mine_d[h][:],
                        in_=xw1my_sb[:, h * HB:h * HB + HB, :])
                    nc.gpsimd.collective_compute(
                        "AllGather", mybir.AluOpType.bypass,
                        replica_groups=groups,
                        ins=[xw1_mine_d[h][:]], outs=[xw1_full_d[h][:]],
                    )
                    for j in range(N_CORES):
                        nc.sync.dma_start(
                            out=xw1_sb[:, j * KPC + h * HB:
                                       j * KPC + h * HB + HB, :],
                            in_=xw1_full_d[h][j])

            # ---- stage 2: H1^T = relu(XW1^T @ A_i^T), feat-major.
            # Half-major k walk keeps 4 PSUM groups open so compute on
            # half 0 hides the half-1 AllGather.
            with tc.tile_pool(name="ps2", bufs=4, space="PSUM") as ps2:
                accs = {}
                for fb in range(D_H // 128):
                    for nb in range(ROWS // 512):
                        accs[fb, nb] = ps2.tile([128, 512], f32, tag="ps",
                                                name=f"acc_h1_{fb}{nb}")
                ks = half_ks(0) + half_ks(1)
                for ki, k in enumerate(ks):
                    for fb in range(D_H // 128):
                        for nb in range(ROWS // 512):
                            nc.tensor.matmul(
                                accs[fb, nb][:],
                                xw1_sb[:, k, fb * 128:(fb + 1) * 128],
                                at_sb[:, k, nb * 512:(nb + 1) * 512],
                                start=(ki == 0),
                                stop=(ki == KT - 1),
                            )
                for fb in range(D_H // 128):
                    for nb in range(ROWS // 512):
                        nc.scalar.activation(
                            h1t_sb[:, fb, nb * 512:(nb + 1) * 512],
                            accs[fb, nb][:],
                            mybir.ActivationFunctionType.Relu, bias=bias_sb[:],
                        )

            # ---- stage 3: Y = H1 @ W2 (node-major) + AllGather (halved) ----
            with tc.tile_pool(name="ps3", bufs=2, space="PSUM") as ps3:
                for h in range(2):
                    for b in range(h * HB, h * HB + HB):
                        acc = ps3.tile([128, D_Z], f32, tag="ps", name="acc_y")
                        for fb in range(D_H // 128):
                            nc.tensor.matmul(
                                acc[:],
                                h1t_sb[:, fb, b * 128:(b + 1) * 128],
                                w2_sb[:, fb, :],
                                start=(fb == 0),
                                stop=(fb == D_H // 128 - 1),
                            )
                        nc.vector.tensor_copy(y_stage[:, b, :], acc[:])
                    nc.scalar.dma_start(
                        out=y_mine_d[h][:],
                        in_=y_stage[:, h * HB:h * HB + HB, :])
                    nc.gpsimd.collective_compute(
                        "AllGather", mybir.AluOpType.bypass,
                        replica_groups=groups,
                        ins=[y_mine_d[h][:]], outs=[y_full_d[h][:]],
                    )
                    for j in range(N_CORES):
                        nc.sync.dma_start(
                            out=y_sb[:, j * KPC + h * HB:
                                     j * KPC + h * HB + HB, :],
                            in_=y_full_d[h][j])

            # ---- stage 4: Z^T = Y^T @ A_i^T (feat-major), AllGather ----
            # Both 512-column halves run as concurrent matmuls on PE
            # column groups 0-1 / 2-3 (out partitions 0-63 / 64-127 of a
            # shared PSUM tile derive the tile_position automatically),
            # since a 64-row output leaves half the array idle otherwise.
            with tc.tile_pool(name="ps4", bufs=1, space="PSUM") as ps4:
                ks = half_ks(0) + half_ks(1)
                acc = ps4.tile([128, 512], f32, tag="ps", name="acc_zt")
                zth_sb = big.tile([128, 512], bf, name="zth_sb")
                for ki, k in enumerate(ks):
                    for nb in range(ROWS // 512):
                        nc.tensor.matmul(
                            acc[nb * D_Z:(nb + 1) * D_Z, :],
                            y_sb[:, k, :],
                            at_sb[:, k, nb * 512:(nb + 1) * 512],
                            start=(ki == 0),
                            stop=(ki == KT - 1),
                        )
                # undo the fp8 A^T pre-scale (Y already absorbed one
                # factor via W2; this is the second A application)
                nc.scalar.activation(
                    ztm_sb[:, 0:512], acc[0:D_Z, :],
                    mybir.ActivationFunctionType.Copy, scale=1.0 / A_SCALE,
                )
                nc.scalar.activation(
                    zth_sb[D_Z:128, :], acc[D_Z:128, :],
                    mybir.ActivationFunctionType.Copy, scale=1.0 / A_SCALE,
                )
                nc.scalar.dma_start(out=zt_mine_d[:, 0:512],
                                    in_=ztm_sb[:, 0:512])
                nc.scalar.dma_start(out=zt_mine_d[:, 512:1024],
                                    in_=zth_sb[D_Z:128, :])
                # partition shift 64->0 so stage 5's stationary operand
                # sees the full [64, 1024] Z^T block
                nc.sync.dma_start(out=ztm_sb[:, 512:1024],
                                  in_=zth_sb[D_Z:128, :])
                nc.gpsimd.collective_compute(
                    "AllGather", mybir.AluOpType.bypass,
                    replica_groups=groups,
                    ins=[zt_mine_d[:]], outs=[zt_full_d[:]],
                )
                for j in range(N_CORES):
                    nc.sync.dma_start(out=ztf_sb[:, j, :], in_=zt_full_d[j])

            # ---- stage 5: recon_i = Z_i @ Z^T, bf16 out.
            # Column chunks walk half 0 of every rank first so the
            # half-1 Z^T AllGather hides behind them.
            ci = 0
            out_dge = [nc.sync, nc.scalar]
            with tc.tile_pool(name="ps5", bufs=4, space="PSUM") as ps5:
                for b in range(MB):
                    ot = outp.tile([128, N_NODES], bf, tag="out", name="out_sb")
                    for s in range(N_NODES // 512):
                        j, off = divmod(s * 512, ROWS)
                        acc = ps5.tile([128, 512], f32, tag="ps",
                                       name="acc_rec")
                        nc.tensor.matmul(
                            acc[:],
                            ztm_sb[:, b * 128:(b + 1) * 128],
                            ztf_sb[:, j, off:off + 512],
                            start=True, stop=True,
                        )
                        copy_to(ci, ot[:, s * 512:(s + 1) * 512], acc[:])
                        ci += 1
                    # rings alternate per row block so writes drain in
                    # parallel on both DGE paths
                    out_dge[b % 2].dma_start(
                        out=recon_out[b * 128:(b + 1) * 128, :],
                        in_=ot[:],
                    )

    nc.finalize()
    return nc


def _prep_inputs(x, edge_src, edge_dst, edge_w, W1, W2):
    """Host-side: dense A^T (fp8), per-core slices, PE-friendly layouts."""
    at = np.zeros((N_NODES, N_NODES), dtype=np.float32)     # [src, dst]
    np.add.at(at, (edge_src, edge_dst), edge_w)
    at *= A_SCALE
    np.minimum(at, 15.5, out=at)     # e3m4 max normal (dup edges can sum)

    xt = np.ascontiguousarray(x.T)                          # [feat, node]
    w1_dev = np.ascontiguousarray(
        W1.reshape(D_IN // 128, 128, D_H).transpose(1, 0, 2)).astype(BF16)
    w2_dev = np.ascontiguousarray(
        (W2 / A_SCALE).reshape(D_H // 128, 128, D_Z).transpose(1, 0, 2)
    ).astype(BF16)

    in_maps = []
    for i in range(N_CORES):
        cols = slice(i * ROWS, (i + 1) * ROWS)
        at_i = np.ascontiguousarray(
            at[:, cols].reshape(KT, 128, ROWS).transpose(1, 0, 2)).astype(E3M4)
        xt_i = np.ascontiguousarray(
            xt[:, cols].reshape(D_IN // 128, 128, ROWS).transpose(1, 0, 2)
        ).astype(BF16)
        in_maps.append({
            "at_in": at_i, "xt_in": xt_i, "w1_in": w1_dev, "w2_in": w2_dev,
        })
    return in_maps


def kernel(x, edge_src, edge_dst, edge_w, W1, W2):
    global _COMPILED, LAST_EXEC_TIME_NS, LAST_RESULTS
    from concourse.bass_utils import run_bass_kernel_spmd

    if _COMPILED is None:
        _COMPILED = _build_program()
    nc = _COMPILED

    in_maps = _prep_inputs(
        np.asarray(x, dtype=np.float32),
        np.asarray(edge_src), np.asarray(edge_dst),
        np.asarray(edge_w, dtype=np.float32),
        np.asarray(W1, dtype=np.float32), np.asarray(W2, dtype=np.float32),
    )

    trace = bool(int(os.environ.get("KERNEL_TRACE", "0")))
    res = run_bass_kernel_spmd(
        nc, in_maps, list(range(N_CORES)), trace=trace,
    )
    LAST_RESULTS = res
    LAST_EXEC_TIME_NS = res.exec_time_ns
    return np.concatenate(
        [res.results[i]["recon"].astype(np.float32) for i in range(N_CORES)],
        axis=0,
    )
